# revision 75
# baseline (speedup 1.0000x reference)
# Self-contained Trainium2 Bass kernel for GQA with sliding-window attention.
#
# Module: B=1, T=2048, C=2048, 32 q-heads / 8 kv-heads, d_head=64, RoPE,
# sliding-window causal attention (window=512), output projection.
#
# Sharding: tensor parallel over heads across 8 cores. Core c owns q-heads
# [4c, 4c+4) and kv-head c, computes attn_out_shard [T, 256] and the partial
# output attn_out_shard @ wo[256c':256(c'+1), :]; host sums the 8 partials.
#
# Layout strategy (all matmuls bf16; x fed pre-transposed + bf16 from
# host so no on-chip transposes of x are needed):
#   - QT/KT = w^T x^T come out of the PE directly in [head_dim-part, t]
#     layout. head_dim is stored in interleaved order [0,32,1,33,...]
#     (scores are invariant to a shared d-permutation of Q and K), which
#     turns RoPE's rotate_half into an adjacent-partition-pair swap done
#     with a DVE stream_shuffle + signed-sin table. The 1/sqrt(d) scale
#     is folded into wq on the host. K and V projections share one
#     matmul (wk|wv concatenated column-wise). RoPE runs fully in bf16
#     SBUF (ACT copies the PSUM result down first) so the DVE ops hit
#     the 2x 16-bit path and the tables halve their DMA footprint.
#   - Scores are computed TRANSPOSED: ST[tk, tq] = matmul(lhsT=KT,
#     rhs=QT) per 128-wide tk chunk, so the post-softmax P needs no
#     transpose for the PV matmul. The 4 older chunks land in a
#     [128,512] PSUM tile; the diagonal chunk lands in the slot's pw
#     PSUM tile (cols 256:512), which keeps every PSUM tile at one bank
#     and frees enough banks for pw to quadruple-buffer. Masking is
#     applied post-exp with affine_select (two triangular chunks) on the
#     mostly-idle Pool engine; out-of-window chunks are never computed.
#   - Softmax denominators come free: V blocks are stored [ones|V|ones]
#     (66 cols); head-0's PV uses the [V|ones] window -> O^T rows 0:64
#     plus an L row at 64, head-1 uses [ones|V] -> L row at 63 plus O^T
#     rows 64:128. No separate ones-vector matmul chain. Normalization =
#     2 reciprocals + a K=65 broadcast matmul + one tensor_mul per head
#     writing attnT.
#   - out = attnT^T @ wo accumulated over the two 128-slices of the 256
#     shard dim; partials written as bf16 and summed in f32 on the host.
#     PSUM->SBUF copies run on ACT so the DVE queue (which frees pw via
#     the normalize muls) stays short.
#   - Single unified PSUM pool scope (8 banks: pa x2, pw x4, st x2);
#     phase B/C run as a 3-stage software pipeline (scores -> PV+recip ->
#     normalize -> out projection) so the PE never waits on the ACT/DVE
#     chains. Input DMA is chunked and priority-ordered so the first
#     matmul starts ~2.5us in and xT superblocks stream ahead of use.

import numpy as np

T = 2048
C = 2048
N_HEADS = 32
N_KV = 8
D = 64
WINDOW = 512
NCORES = 8
HQ = N_HEADS // NCORES          # 4 q heads per core
OQ = HQ * D                     # 256
ROPE_BASE = 10000.0
SCALE = 1.0 / 8.0               # 1/sqrt(64), folded into wq host-side
NB = T // 128                   # 16 row blocks
NS = T // 512                   # 4 superblocks
VW = D + 65                     # 129: [zeros*32 | ones | zeros*31 | V |
                                # ones]; the zero pad lets head-1's PV chain
                                # output at base partition 0 with M=128 (HW
                                # restricts matmul out base/span to
                                # quadrants) and puts its L row at partition
                                # 32 (engine APs need 32-aligned bases)

_cache = {}


def _dperm():
    # Interleaved head_dim order [0,32,1,33,...]: rotate_half becomes an
    # adjacent-pair swap, expressible as a DVE stream_shuffle. Scores are
    # invariant to any d-permutation applied to both Q and K.
    pi = np.empty(D, dtype=np.int64)
    pi[0::2] = np.arange(D // 2)
    pi[1::2] = np.arange(D // 2) + D // 2
    return pi


def _rope_tables():
    inv = 1.0 / (ROPE_BASE ** (np.arange(0, D, 2, dtype=np.float64) / D))
    t = np.arange(T, dtype=np.float64)
    fr = t[:, None] * inv[None, :]            # [T, 32]
    emb = np.concatenate([fr, fr], axis=1)    # [T, 64]
    cos = np.cos(emb).T.astype(np.float32)    # [64, T]
    sin = np.sin(emb).T.astype(np.float32)
    sinS = sin.copy()
    sinS[: D // 2] *= -1.0                    # signed sin for rotate_half
    pi = _dperm()
    cos = cos[pi]
    sinS = sinS[pi]
    cos2 = np.concatenate([cos, cos], axis=0)     # [128, T] (2 heads/tile)
    sinS2 = np.concatenate([sinS, sinS], axis=0)  # [128, T]
    return cos2, sinS2


def _e2():
    # Selector for the Linv broadcast matmul (lhsT, [65,128]).
    # Partition 32 holds head-1's Linv -> broadcast to out rows 64:128;
    # partition 64 holds head-0's Linv -> out rows 0:64; rest contracts
    # against zeroed rows of the rl tile.
    e = np.zeros((65, 128), dtype=np.float32)
    e[32, 64:128] = 1.0
    e[64, 0:64] = 1.0
    return e


def _build():
    import concourse.bacc as bacc
    import concourse.mybir as mybir
    import concourse.tile as tile

    f32 = mybir.dt.float32
    bf16 = mybir.dt.bfloat16
    EXP = mybir.ActivationFunctionType.Exp
    GE = mybir.AluOpType.is_ge

    nc = bacc.Bacc("TRN2", target_bir_lowering=False, debug=False,
                   num_devices=NCORES)

    # xT4[s] = x^T columns for superblock s, chunked: [128, 16*512]
    xT_d = nc.dram_tensor("xT4", [NS, 128, 16 * 512], bf16,
                          kind="ExternalInput").ap()
    # superblock 0 again, token-block-major: [128, b*2048 + cc*128 + tt]
    # so phase A(0) can run per 128-token block, interleaved with the
    # first attention slots while the input stream is still arriving
    xT0b_d = nc.dram_tensor("xT0b", [128, 4 * 16 * 128], bf16,
                            kind="ExternalInput").ap()
    wq_d = nc.dram_tensor("wqT", [128, 16 * OQ], bf16,
                          kind="ExternalInput").ap()
    wkv_d = nc.dram_tensor("wkvT", [128, 16 * 128], bf16,
                           kind="ExternalInput").ap()
    wo_d = nc.dram_tensor("woT", [128, 2 * C], bf16,
                          kind="ExternalInput").ap()
    cos_d = nc.dram_tensor("cos2", [128, T], bf16, kind="ExternalInput").ap()
    sin_d = nc.dram_tensor("sinS2", [128, T], bf16, kind="ExternalInput").ap()
    e2_d = nc.dram_tensor("e2", [65, 128], bf16, kind="ExternalInput").ap()
    out_d = nc.dram_tensor("out", [T, C], bf16, kind="ExternalOutput").ap()

    with tile.TileContext(nc) as tc:
        from contextlib import ExitStack
        ctx = ExitStack()
        with ctx:
            const = ctx.enter_context(tc.tile_pool(name="const", bufs=1))
            persist = ctx.enter_context(tc.tile_pool(name="persist", bufs=1))

            # ---- constants / weights into SBUF ----
            from concourse.masks import make_identity
            identb = const.tile([128, 128], bf16, tag="identb", name="identb")
            make_identity(nc, identb[:])

            wq_sb = const.tile([128, 16 * OQ], bf16, tag="wq", name="wq")
            wkv_sb = const.tile([128, 16 * 128], bf16, tag="wkv", name="wkv")
            xT_sb = [None] + [const.tile([128, 16 * 512], bf16, tag=f"xT{s}",
                              name=f"xT{s}") for s in range(1, NS)]
            xT0b_sb = const.tile([128, 4 * 16 * 128], bf16, tag="xT0b",
                                 name="xT0b")
            cos2 = const.tile([128, T], bf16, tag="cos2", name="cos2")
            sinS2 = const.tile([128, T], bf16, tag="sinS2", name="sinS2")
            e2_sb = const.tile([65, 128], bf16, tag="e2", name="e2")
            wo_sb = const.tile([128, 2 * C], bf16, tag="wo", name="wo")

            # Priority-ordered, chunked input DMA, ALL on one queue (SP):
            # the shared DMA engines serve whichever queue's descriptor
            # stage finishes first, so multi-queue issue does not preserve
            # priority — a single queue does. Stream order mirrors
            # consumption: wq/xT0 stripes (A(0,g0/g1)), rope tables for
            # superblock 0 only (128KB each; A(s) uses cols s*512 only),
            # wkv halves, then xT1 before A(1) at slot 5, wo before the
            # first C at slot 11, then xT2/xT3 + remaining table chunks.
            nc.sync.dma_start(out=wq_sb[:, 0:2 * OQ], in_=wq_d[:, 0:2 * OQ])
            nc.sync.dma_start(out=xT0b_sb[:, 0:512], in_=xT0b_d[:, 0:512])
            nc.sync.dma_start(out=xT0b_sb[:, 512:2048],
                              in_=xT0b_d[:, 512:2048])
            nc.sync.dma_start(out=wq_sb[:, 2 * OQ:8 * OQ],
                              in_=wq_d[:, 2 * OQ:8 * OQ])
            nc.sync.dma_start(out=wq_sb[:, 8 * OQ:16 * OQ],
                              in_=wq_d[:, 8 * OQ:16 * OQ])
            nc.sync.dma_start(out=wkv_sb[:, 0:1024], in_=wkv_d[:, 0:1024])
            nc.sync.dma_start(out=wkv_sb[:, 1024:2048],
                              in_=wkv_d[:, 1024:2048])
            nc.sync.dma_start(out=cos2[:, 0:512], in_=cos_d[:, 0:512])
            nc.sync.dma_start(out=sinS2[:, 0:512], in_=sin_d[:, 0:512])
            nc.sync.dma_start(out=xT0b_sb[:, 2048:4096],
                              in_=xT0b_d[:, 2048:4096])
            nc.sync.dma_start(out=xT0b_sb[:, 4096:6144],
                              in_=xT0b_d[:, 4096:6144])
            nc.sync.dma_start(out=xT0b_sb[:, 6144:8192],
                              in_=xT0b_d[:, 6144:8192])
            for q in range(4):
                nc.sync.dma_start(out=xT_sb[1][:, q * 2048:(q + 1) * 2048],
                                  in_=xT_d[1][:, q * 2048:(q + 1) * 2048])
            nc.sync.dma_start(out=cos2[:, 512:1024], in_=cos_d[:, 512:1024])
            nc.sync.dma_start(out=sinS2[:, 512:1024],
                              in_=sin_d[:, 512:1024])
            nc.sync.dma_start(out=e2_sb[:, :], in_=e2_d[:, :])
            nc.sync.dma_start(out=wo_sb[:, 0:C], in_=wo_d[:, 0:C])
            nc.sync.dma_start(out=wo_sb[:, C:2 * C], in_=wo_d[:, C:2 * C])
            for q in range(4):
                nc.sync.dma_start(out=xT_sb[2][:, q * 2048:(q + 1) * 2048],
                                  in_=xT_d[2][:, q * 2048:(q + 1) * 2048])
            nc.sync.dma_start(out=cos2[:, 1024:1536],
                              in_=cos_d[:, 1024:1536])
            nc.sync.dma_start(out=sinS2[:, 1024:1536],
                              in_=sin_d[:, 1024:1536])
            for q in range(4):
                nc.sync.dma_start(out=xT_sb[3][:, q * 2048:(q + 1) * 2048],
                                  in_=xT_d[3][:, q * 2048:(q + 1) * 2048])
            nc.sync.dma_start(out=cos2[:, 1536:2048],
                              in_=cos_d[:, 1536:2048])
            nc.sync.dma_start(out=sinS2[:, 1536:2048],
                              in_=sin_d[:, 1536:2048])

            # ---- persistent activations ----
            QTr = [persist.tile([128, T], bf16, tag=f"QTr{hp}",
                                name=f"QTr{hp}") for hp in range(2)]
            KTr = persist.tile([128, T], bf16, tag="KTr", name="KTr")
            # VB: [zeros*32 | ones | zeros*31 | V | ones] per tk block
            VB = persist.tile([128, NB * VW], bf16, tag="VB", name="VB")
            # attnT as per-block tiles: a C-proj lhsT read then depends
            # only on its own block's normalize, not on every later
            # normalize write into one big tile.
            attnT = [[persist.tile([128, 128], bf16, tag=f"attnT{oc}_{b}",
                                   name=f"attnT{oc}_{b}")
                      for b in range(NB)] for oc in range(2)]
            nc.gpsimd.memset(VB[:, :], 0.0)
            for blk in range(NB):
                nc.gpsimd.memset(VB[:, blk * VW + 32:blk * VW + 33], 1.0)
                nc.gpsimd.memset(VB[:, blk * VW + 128:blk * VW + 129], 1.0)
            # rl tiles: rows 63/64 get per-head recip sums; rest stay 0
            # so the K=65 broadcast matmul contracts against zeros.
            rls = [persist.tile([65, 128], bf16, tag=f"rl{k}", name=f"rl{k}")
                   for k in range(3)]
            for r_ in rls:
                nc.gpsimd.memset(r_[:, :], 0.0)

            # ===== Unified pipeline: A (proj/RoPE/V) interleaved with =====
            # ===== B (attention) and C (output projection)            =====
            # PSUM (8 banks): pa [128,512]f32 x2 (C-proj halves + proj
            # tiles; the half-per-slot C cadence gives each psc copy a
            # slot of slack), pb [128,512]f32 x4 (pw: PV out + diag
            # scores + Linv; 4-deep so normalize runs at slot k+3 and
            # recips get two slots), ps [128,512]f32 x2 (older score
            # chunks + vp/kd).
            with tc.tile_pool(name="pa", bufs=3, space="PSUM") as pa, \
                 tc.tile_pool(name="pb", bufs=3, space="PSUM") as pb, \
                 tc.tile_pool(name="ps", bufs=2, space="PSUM") as ps, \
                 tc.tile_pool(name="tmpA", bufs=2) as tmpA, \
                 tc.tile_pool(name="outp", bufs=3) as outp, \
                 tc.tile_pool(name="tmpB", bufs=5) as tmpB:

                SWAP_PAIRS = [i ^ 1 for i in range(32)]

                def rope_pre(psrc, P, W=512):
                    # PSUM -> bf16 SBUF on ACT; frees the proj PSUM tile
                    # promptly and lets the DVE part run all-16-bit.
                    qsb = tmpA.tile([128, 512], bf16, tag="rope_q",
                                    name="rope_q")
                    nc.scalar.copy(qsb[:P, :W], psrc[:P, :W])
                    return qsb

                def rope_dve(qsb, P, dst, scol, W=512):
                    # dst = q*cos + shuffle(q)*sinS (interleaved-d layout
                    # makes rotate_half an adjacent-pair swap).
                    t1 = tmpA.tile([128, 512], bf16, tag="rope_t1",
                                   name="rope_t1")
                    nc.vector.tensor_mul(t1[:P, :W], qsb[:P, :W],
                                         cos2[:P, scol:scol + W])
                    sh = tmpA.tile([128, 512], bf16, tag="rope_sh",
                                   name="rope_sh")
                    nc.vector.stream_shuffle(sh[:P, :W], qsb[:P, :W],
                                             SWAP_PAIRS)
                    t2 = tmpA.tile([128, 512], bf16, tag="rope_t2",
                                   name="rope_t2")
                    nc.vector.tensor_mul(t2[:P, :W], sh[:P, :W],
                                         sinS2[:P, scol:scol + W])
                    nc.vector.tensor_add(dst, t1[:P, :W], t2[:P, :W])

                def emit_a_group(s, g):
                    scol = s * 512
                    if g < 2:
                        ob = g
                        psq = pa.tile([128, 512], f32, tag="pa", name="psq")
                        for cc in range(16):
                            nc.tensor.matmul(
                                psq[:],
                                lhsT=wq_sb[:, cc * OQ + ob * 128:
                                           cc * OQ + (ob + 1) * 128],
                                rhs=xT_sb[s][:, cc * 512:(cc + 1) * 512],
                                start=(cc == 0), stop=(cc == 15))
                        qsb = rope_pre(psq, 128)
                        # the DVE part is deferred past this slot's
                        # latency-critical recip/normalize DVE work
                        return (lambda: rope_dve(
                            qsb, 128, QTr[ob][:, scol:scol + 512], scol),
                            None)
                    # K (rows 0:64) and V (rows 64:128) in one matmul
                    pskv = pa.tile([128, 512], f32, tag="pa", name="pskv")
                    for cc in range(16):
                        nc.tensor.matmul(
                            pskv[:],
                            lhsT=wkv_sb[:, cc * 128:(cc + 1) * 128],
                            rhs=xT_sb[s][:, cc * 512:(cc + 1) * 512],
                            start=(cc == 0), stop=(cc == 15))
                    qsb = rope_pre(pskv, 64)
                    # K rope runs inline: the kd duplication and the next
                    # superblock's scores need KTr as soon as possible
                    rope_dve(qsb, 64, KTr[:64, scol:scol + 512], scol)

                    def tail():
                        # V -> bf16 (deferred: keeps the ACT queue clear
                        # for this slot's exps), then the K-duplication
                        # and V transposes; intervening attention matmuls
                        # keep the in-order PE queue busy meanwhile.
                        vtsb = tmpA.tile([128, 512], bf16, tag="vtsb",
                                         name="vtsb")
                        for b in range(4):
                            nc.scalar.copy(
                                vtsb[64:128, b * 128:(b + 1) * 128],
                                pskv[64:128, b * 128:(b + 1) * 128])
                        # duplicate K rows for the hh=1 partition-aligned
                        # matmuls via identity matmul (a DMA here would
                        # queue behind the whole input stream on the
                        # shared DMA engines and stall hh=1 scores)
                        kd = ps.tile([128, 512], f32, tag="ps", name="kd")
                        nc.tensor.matmul(kd[64:128, :],
                                         lhsT=identb[0:64, 0:64],
                                         rhs=KTr[0:64, scol:scol + 512],
                                         start=True, stop=True)
                        nc.vector.tensor_copy(
                            KTr[64:128, scol:scol + 512], kd[64:128, :])
                        # transpose each 128-block of V to [t, d]
                        for b in range(4):
                            vp = ps.tile([128, 64], bf16, tag="ps",
                                         name="vp")
                            nc.tensor.transpose(
                                vp[:], vtsb[64:128, b * 128:(b + 1) * 128],
                                identb[64:128, 64:128])
                            blk = s * 4 + b
                            nc.scalar.copy(
                                VB[:, blk * VW + 64:blk * VW + 64 + D],
                                vp[:])
                    return None, tail

                def emit_a0_block(b):
                    # Phase A for superblock 0, one 128-token block at a
                    # time (token-major xT0b layout): lets the first
                    # attention slots interleave into the DMA-paced
                    # startup window. Returns the deferred tail.
                    col = b * 128
                    for g in range(2):
                        psq = pa.tile([128, 512], f32, tag="pa",
                                      name="psq0")
                        for cc in range(16):
                            nc.tensor.matmul(
                                psq[:, 0:128],
                                lhsT=wq_sb[:, cc * OQ + g * 128:
                                           cc * OQ + (g + 1) * 128],
                                rhs=xT0b_sb[:, b * 2048 + cc * 128:
                                            b * 2048 + (cc + 1) * 128],
                                start=(cc == 0), stop=(cc == 15))
                        qsb = rope_pre(psq, 128, 128)
                        rope_dve(qsb, 128, QTr[g][:, col:col + 128],
                                 col, 128)
                    pskv = pa.tile([128, 512], f32, tag="pa", name="pskv0")
                    for cc in range(16):
                        nc.tensor.matmul(
                            pskv[:, 0:128],
                            lhsT=wkv_sb[:, cc * 128:(cc + 1) * 128],
                            rhs=xT0b_sb[:, b * 2048 + cc * 128:
                                        b * 2048 + (cc + 1) * 128],
                            start=(cc == 0), stop=(cc == 15))
                    qsb = rope_pre(pskv, 64, 128)
                    rope_dve(qsb, 64, KTr[:64, col:col + 128], col, 128)

                    def tail():
                        vtsb = tmpA.tile([128, 128], bf16, tag="vtsb0",
                                         name="vtsb0")
                        nc.scalar.copy(vtsb[64:128, :], pskv[64:128, 0:128])
                        kd = ps.tile([128, 512], f32, tag="ps", name="kd")
                        nc.tensor.matmul(kd[64:128, 0:128],
                                         lhsT=identb[0:64, 0:64],
                                         rhs=KTr[0:64, col:col + 128],
                                         start=True, stop=True)
                        nc.vector.tensor_copy(
                            KTr[64:128, col:col + 128], kd[64:128, 0:128])
                        vp = ps.tile([128, 64], bf16, tag="ps", name="vp")
                        nc.tensor.transpose(vp[:], vtsb[64:128, :],
                                            identb[64:128, 64:128])
                        nc.scalar.copy(
                            VB[:, b * VW + 64:b * VW + 64 + D], vp[:])
                    return tail

                def normalize(i, hp, pw):
                    # Linv broadcast + per-head normalize writing attnT.
                    # (reciprocals were emitted at the end of the producing
                    # slot) The muls read the Linv broadcast straight from
                    # the same PSUM tile (cols 384:512) - no staging copy.
                    qcol = i * 128
                    rl = rls[(i * 2 + hp) % 3]
                    nc.tensor.matmul(pw[:, 384:512], lhsT=e2_sb[:, :],
                                     rhs=rl[:, :], start=True, stop=True)
                    linb = tmpB.tile([128, 128], f32, tag="linb",
                                     name="linb")
                    nc.vector.tensor_copy(linb[:, :], pw[:, 384:512])
                    nc.vector.tensor_mul(
                        attnT[hp][i][0:64, :],
                        pw[0:64, 0:128], linb[0:64, :])
                    nc.vector.tensor_mul(
                        attnT[hp][i][64:128, :],
                        pw[64:128, 128:256], linb[64:128, :])

                osb_cur = [None]

                def emit_c_half(tb, half, split=False):
                    # output projection for finished row block tb, two
                    # 512-col chunks per call (a full C-proj in one slot
                    # starves the pa ring against its own copies; spread
                    # over two slots the copies have a slot of slack).
                    # Copies alternate DVE/ACT and are emitted last in
                    # the slot so they never delay recips/exps.
                    if half == 0:
                        osb_cur[0] = outp.tile([128, C], bf16, tag="osb",
                                               name="osb")
                    osb = osb_cur[0]
                    cpeng = [nc.vector.tensor_copy, nc.scalar.copy]
                    for h in range(2):
                        cr = half * 2 + h
                        op = pa.tile([128, 512], f32, tag="pa", name="psc")
                        for oc in range(2):
                            nc.tensor.matmul(
                                op[:],
                                lhsT=attnT[oc][tb][:, :],
                                rhs=wo_sb[:, oc * C + cr * 512:
                                          oc * C + (cr + 1) * 512],
                                start=(oc == 0), stop=(oc == 1))
                        cpeng[h](osb[:, cr * 512:(cr + 1) * 512], op[:])
                    if split:
                        # drain path: SWDGE pairs - descriptor prep runs
                        # ahead on the idle Pool engine, so after the
                        # copy only transfer+semaphore remain
                        nc.gpsimd.dma_start(
                            out=out_d[tb * 128:(tb + 1) * 128,
                                      half * 1024:(half + 1) * 1024],
                            in_=osb[:, half * 1024:(half + 1) * 1024])
                    elif half == 1:
                        nc.gpsimd.dma_start(
                            out=out_d[tb * 128:(tb + 1) * 128, :], in_=osb[:])

                def emit_c(tb, split=False):
                    emit_c_half(tb, 0, split)
                    emit_c_half(tb, 1, split)

                # Attention work tile pw [128,512] col map: 0:128 h0
                # [O^T;L] rows 0:65 (VB ones col), 128:256 h1 [L;O^T] rows
                # 63:128, 256:384 hh0 diagonal scores chunk, 384:512 hh1
                # diagonal scores chunk then (post-exp) Linv broadcast.
                # 3-stage software pipeline per slot k:
                #   STs(k) -> PVs(k-1)+recips -> normalize(k-2) -> C
                # so the PE never waits on exp/affine or the DVE chain.
                slots = [(i, hp) for i in range(NB) for hp in range(2)]
                pend_pv = []      # [(k, pts, pw)]
                pend_norm = []    # [(i, hp, pw)]

                def emit_scores(k):
                    i, hp = slots[k]
                    j0 = max(0, 4 - i)
                    c0 = j0 * 128
                    qcol = i * 128
                    pw = pb.tile([128, 512], f32, tag="pb", name="pw")
                    pts = []
                    for hh in range(2):
                        hoff = hh * 64
                        qs = QTr[hp][hoff:hoff + 64, qcol:qcol + 128]
                        st = None
                        if j0 < 4:
                            st = ps.tile([128, 512], f32, tag="ps",
                                         name="st")
                            for j in range(j0, 4):
                                tkb = i - 4 + j
                                nc.tensor.matmul(
                                    st[:, j * 128:(j + 1) * 128],
                                    lhsT=KTr[hoff:hoff + 64,
                                             tkb * 128:(tkb + 1) * 128],
                                    rhs=qs, start=True, stop=True)
                        # diagonal chunk into pw cols 256+hh*128
                        dcol = 256 + hh * 128
                        nc.tensor.matmul(
                            pw[:, dcol:dcol + 128],
                            lhsT=KTr[hoff:hoff + 64,
                                     i * 128:(i + 1) * 128],
                            rhs=qs, start=True, stop=True)
                        pt = tmpB.tile([128, 640], bf16, tag="pt",
                                       name="pt")
                        if j0 < 4:
                            nc.scalar.activation(pt[:, c0:512],
                                                 st[:, c0:512], EXP)
                        nc.scalar.activation(pt[:, 512:640],
                                             pw[:, dcol:dcol + 128], EXP)
                        # diagonal chunk: keep tk<=tq (p <= col)
                        nc.gpsimd.affine_select(
                            out=pt[:, 512:640], in_=pt[:, 512:640],
                            compare_op=GE, fill=0.0, base=0,
                            pattern=[[1, 128]], channel_multiplier=-1)
                        if i >= 4:
                            # oldest chunk: keep tq-tk<=512 (p >= col)
                            nc.gpsimd.affine_select(
                                out=pt[:, 0:128], in_=pt[:, 0:128],
                                compare_op=GE, fill=0.0, base=0,
                                pattern=[[-1, 128]], channel_multiplier=1)
                        pts.append(pt)
                    pend_pv.append((k, pts, pw))

                def emit_pv():
                    k, pts, pw = pend_pv.pop(0)
                    i, hp = slots[k]
                    j0 = max(0, 4 - i)
                    # masked chunks (affine-gated) go last in each chain
                    if i >= 4:
                        js = [1, 2, 3, 0, 4]
                    else:
                        js = list(range(j0, 5))
                    for hh in range(2):
                        for n_, j in enumerate(js):
                            tkb = i - 4 + j
                            pcol = pts[hh][:, j * 128:(j + 1) * 128]
                            first = n_ == 0
                            last = n_ == len(js) - 1
                            if hh == 0:
                                # lhsT = [V|ones] -> rows 0:64 O^T, row 64 L
                                nc.tensor.matmul(
                                    pw[0:65, 0:128],
                                    lhsT=VB[:, tkb * VW + 64:(tkb + 1) * VW],
                                    rhs=pcol,
                                    start=first, stop=last)
                            else:
                                # lhsT = [zeros32|ones|zeros31|V] at base 0
                                # -> row 32 L, rows 64:128 O^T, rest zero
                                nc.tensor.matmul(
                                    pw[0:128, 128:256],
                                    lhsT=VB[:, tkb * VW:tkb * VW + 128],
                                    rhs=pcol,
                                    start=first, stop=last)
                    rl = rls[(i * 2 + hp) % 3]
                    with nc.allow_low_precision(
                            reason="f32r is bit-identical to f32"):
                        nc.vector.reciprocal(rl[64:65, :], pw[64:65, 0:128])
                        nc.vector.reciprocal(rl[32:33, :], pw[32:33, 128:256])
                    pend_norm.append((i, hp, pw))

                # A(0) runs up front (slots 0..7 need only superblock 0);
                # A(s>=1) interleaves into slots 8(s-1)+{5,6,7}, finishing
                # just before slot 8s needs superblock s. This caps the
                # input DMA needed before compute saturates at ~4MB and
                # keeps the PE hot (pstate) through the whole A phase.
                amap = {8 * (s_ - 1) + 5 + g_: (s_, g_)
                        for s_ in range(1, NS) for g_ in range(3)}
                # A(0) runs block-wise, two blocks ahead of the slots
                # that consume them; a0tails[j] must run before slot 2j.
                a0tails = {}
                # C emission starts at k=11 (after wo arrives; an earlier
                # C would stall the in-order PE queue on the wo load),
                # catches up two halves per slot, then one half per slot.
                next_h = 0
                pending_tail = None
                for k in range(len(slots)):
                    if k == 0:
                        a0tails[0] = emit_a0_block(0)
                        a0tails[1] = emit_a0_block(1)
                    elif k == 2:
                        a0tails[2] = emit_a0_block(2)
                    elif k == 4:
                        a0tails[3] = emit_a0_block(3)
                    if k % 2 == 0 and k // 2 in a0tails:
                        a0tails.pop(k // 2)()
                    if pending_tail is not None:
                        pending_tail()
                        pending_tail = None
                    ropefn = None
                    if k in amap:
                        ropefn, t_ = emit_a_group(*amap[k])
                        if t_ is not None:
                            pending_tail = t_
                    emit_scores(k)
                    if pend_pv and len(pend_pv) > 1:
                        emit_pv()
                    if len(pend_norm) > 1:
                        ni, nhp, npw = pend_norm.pop(0)
                        normalize(ni, nhp, npw)
                    if ropefn is not None:
                        # Q-rope DVE chain lands after the critical
                        # recip/normalize DVE work of this slot
                        ropefn()
                    if k >= 11:
                        for _ in range(2):
                            if (next_h < 2 * (NB - 3)
                                    and next_h // 2 <= (k - 7) // 2):
                                emit_c_half(next_h // 2, next_h % 2)
                                behind = next_h // 2 < (k - 7) // 2
                                next_h += 1
                                if not behind:
                                    break
                # drain: C(13)/C(14) fill the PE while the last norms run
                while pend_pv:
                    emit_pv()
                ni, nhp, npw = pend_norm.pop(0)
                normalize(ni, nhp, npw)
                while next_h < 2 * (NB - 3):
                    emit_c_half(next_h // 2, next_h % 2)
                    next_h += 1
                emit_c(NB - 3)
                emit_c(NB - 2, split=True)
                ni, nhp, npw = pend_norm.pop(0)
                normalize(ni, nhp, npw)
                emit_c(NB - 1, split=True)

    nc.compile()
    return nc


def _get_nc():
    if "nc" not in _cache:
        _cache["nc"] = _build()
    return _cache["nc"]


def _host_inputs(x, wq, wk, wv, wo):
    import ml_dtypes
    bf = ml_dtypes.bfloat16
    x2 = np.asarray(x, dtype=np.float32).reshape(T, C)
    # xT4[s, p, cc*512 + tt] = x[s*512 + tt, cc*128 + p]
    xT4 = np.ascontiguousarray(
        x2.reshape(NS, 512, 16, 128).transpose(0, 3, 2, 1)
        .reshape(NS, 128, 16 * 512)).astype(bf)
    # xT0b[p, b*2048 + cc*128 + tt] = x[b*128 + tt, cc*128 + p]
    xT0b = np.ascontiguousarray(
        x2[0:512].reshape(4, 128, 16, 128).transpose(3, 0, 2, 1)
        .reshape(128, 4 * 16 * 128)).astype(bf)
    cos2, sinS2 = _rope_tables()
    cos2 = cos2.astype(bf)
    sinS2 = sinS2.astype(bf)
    pi = _dperm()
    e2 = _e2().astype(bf)
    in_maps = []
    for c in range(NCORES):
        wq_s = (np.asarray(wq[:, c * OQ:(c + 1) * OQ], dtype=np.float32)
                * SCALE)
        wq_s = wq_s.reshape(C, HQ, D)[:, :, pi].reshape(C, OQ)
        wq_t = np.ascontiguousarray(
            wq_s.reshape(16, 128, OQ).transpose(1, 0, 2)
            .reshape(128, 16 * OQ)).astype(bf)
        wk_s = np.asarray(wk[:, c * D:(c + 1) * D], dtype=np.float32)[:, pi]
        wkv = np.concatenate(
            [wk_s,
             np.asarray(wv[:, c * D:(c + 1) * D], dtype=np.float32)],
            axis=1)
        wkv_t = np.ascontiguousarray(
            wkv.reshape(16, 128, 128).transpose(1, 0, 2)
            .reshape(128, 16 * 128)).astype(bf)
        wo_s = np.asarray(wo[c * OQ:(c + 1) * OQ, :], dtype=np.float32)
        wo_t = np.ascontiguousarray(
            wo_s.reshape(2, 128, C).transpose(1, 0, 2)
            .reshape(128, 2 * C)).astype(bf)
        in_maps.append({
            "xT4": xT4,
            "xT0b": xT0b,
            "wqT": wq_t,
            "wkvT": wkv_t,
            "woT": wo_t,
            "cos2": cos2,
            "sinS2": sinS2,
            "e2": e2,
        })
    return in_maps


def kernel(x, wq, wk, wv, wo):
    from concourse.bass_utils import run_bass_kernel_spmd

    nc = _get_nc()
    in_maps = _host_inputs(x, wq, wk, wv, wo)
    res = run_bass_kernel_spmd(nc, in_maps, list(range(NCORES)))
    out = np.zeros((T, C), dtype=np.float32)
    for r in res.results:
        out += np.asarray(r["out"], dtype=np.float32)
    return out.reshape(1, T, C)
